# revision 17
# baseline (speedup 1.0000x reference)
"""UR-LSTM forward kernel for Trainium2 (8 NeuronCores).

Strategy (sequence-parallel with warmup):
  The UR-LSTM state is strongly contractive (forget gates bounded away from
  1), so a chunk of the sequence can be computed exactly (to fp32 noise) by
  starting W steps earlier from zero state.  T=1024 is split into 16 chunks;
  each of the 8 cores runs 2 independent chains.  Every chain runs
  S = C + W steps; the first W steps of chunks 1..15 are discarded warmup.

  Per step, per chain (B=128 full batch on every core):
    gates[2048, 128] = sum_k WtileT[k].T @ state_chunk[k]   (PE, bf16)
      where the contraction is over [h(512); x_t(10); 1; 0-pad] = 5 K-chunks
      of 128.  Bias b and the UR-LSTM fb offsets are folded into the ones-row
      column, so PSUM holds pre-activations directly.
    Gate blocks are laid out [f r o u] so that one wide sigmoid covers
    f,r,o (PSUM tile [128,1536]) and one tanh covers u ([128,512]).
    Elementwise runs in bf16 (2x DVE mode) with fp32 cell state, split in
    halves so h streams out in 256-col pieces; the next step's matmuls are
    ordered k-outer so they start as soon as the first h half lands.
    y_t = W_out @ h_t + b_out is 5 tiny matmuls, lagged one half-round so
    they never block the PE on the EW tail; 4 steps of y accumulate in one
    PSUM bank before a single copy + DMA.
"""

import numpy as np
import ml_dtypes

B, T, I, H = 128, 1024, 10, 512
G4 = 4 * H  # 2048
NCORES = 8
NCHUNK = 16
W_WARM = 16
C_OUT = (T - W_WARM) // NCHUNK  # 63
S_STEPS = C_OUT + W_WARM  # 79
KCH = 5  # 4 h-chunks + 1 (x | ones | pad) chunk
GT = 16  # gate tiles of 128

_cache = {}


def _build_nc(S):
    import concourse.bacc as bacc
    import concourse.mybir as mybir
    import concourse.tile as tile

    dt = mybir.dt
    f32, bf16 = dt.float32, dt.bfloat16
    AF = mybir.ActivationFunctionType
    OP = mybir.AluOpType

    nc = bacc.Bacc(None, target_bir_lowering=False)

    w_d = nc.dram_tensor("w", [128, KCH * GT * 128], bf16, kind="ExternalInput")
    wy_d = nc.dram_tensor("wy", [128, KCH * 10], bf16, kind="ExternalInput")
    x_d = [
        nc.dram_tensor(f"x{c}", [128, S * 128], bf16, kind="ExternalInput")
        for c in range(2)
    ]
    # y blocks ordered g = 2*s + c (step-major, chain-minor)
    y_d = nc.dram_tensor("y", [10, 2 * S * 128], f32, kind="ExternalOutput")

    NYB = 2 * S  # total y blocks of 128

    with tile.TileContext(nc) as tc:
        with (
            tc.tile_pool(name="const", bufs=1) as const,
            tc.tile_pool(name="hpool", bufs=2) as hpool,
            tc.tile_pool(name="ew", bufs=2) as ew,
            tc.tile_pool(name="fro_ps", bufs=2, space="PSUM") as fro_ps,
            tc.tile_pool(name="u_ps", bufs=2, space="PSUM") as u_ps,
            tc.tile_pool(name="yout", bufs=2) as youtp,
        ):
            wbuf = const.tile([128, KCH * GT * 128], bf16, tag="wbuf")
            nc.sync.dma_start(wbuf[:], w_d[:])
            wybuf = const.tile([128, KCH * 10], bf16, tag="wybuf")
            nc.sync.dma_start(wybuf[:], wy_d[:])
            xb = []
            XQ = (S * 128) // 4
            for c in range(2):
                t = const.tile([128, S * 128], bf16, tag=f"xb{c}")
                for q in range(4):
                    nc.sync.dma_start(
                        t[:, q * XQ : (q + 1) * XQ], x_d[c][:, q * XQ : (q + 1) * XQ]
                    )
                xb.append(t)

            # persistent state: c as fp32 halves, h as bf16 halves
            cbuf = []
            h_prev = []
            for c in range(2):
                ch = []
                hh = []
                for half in range(2):
                    ct = const.tile([128, 256], f32, tag=f"c{c}h{half}", name=f"c{c}h{half}")
                    nc.vector.memset(ct[:], 0.0)
                    ch.append(ct)
                    ht = hpool.tile([128, 256], bf16, tag=f"h{c}h{half}", name=f"h{c}h{half}")
                    nc.vector.memset(ht[:], 0.0)
                    hh.append(ht)
                cbuf.append(ch)
                h_prev.append(hh)

            def h_chunk(c, k):
                return h_prev[c][k // 2][:, (k % 2) * 128 : (k % 2 + 1) * 128]

            def x_chunk(c, s):
                return xb[c][:, s * 128 : (s + 1) * 128]

            def w_tile(k, gt):
                return wbuf[:, (k * GT + gt) * 128 : (k * GT + gt + 1) * 128]

            # y state: (pending_h, pending_x_slice, yblock_idx, u_tile) — the
            # y projection lands in a corner of its step's u PSUM tile after
            # tanh(u) has consumed it, so no dedicated y bank is needed
            pend_y = []

            def emit_y_mms(h_halves, xs, g, ut_old):
                out = ut_old[0:10, 384:512]
                for k in range(KCH):
                    rhs = (
                        h_halves[k // 2][:, (k % 2) * 128 : (k % 2 + 1) * 128]
                        if k < 4
                        else xs
                    )
                    nc.tensor.matmul(
                        out,
                        lhsT=wybuf[:, k * 10 : (k + 1) * 10],
                        rhs=rhs,
                        start=(k == 0),
                        stop=(k == KCH - 1),
                    )
                yo = youtp.tile([10, 128], f32, tag="yo", name="yo")
                nc.scalar.copy(yo[:], out)
                nc.sync.dma_start(y_d[:, g * 128 : (g + 1) * 128], yo[:])

            # deferred tail (tanh(c) + h-mult) of the previous chain-step,
            # emitted between the next block's head acts so the scalar FIFO
            # order becomes: sf, sr, tanh0_prev, tanh1_prev, tanh_u, so
            pend_tail = []

            def emit_tail():
                if not pend_tail:
                    return
                c, cb0, cb1, og, h_new = pend_tail.pop(0)
                tc0 = ew.tile([128, 256], bf16, tag=f"tc0{c}", name="tc0")
                nc.scalar.activation(tc0[:], cb0[:], AF.Tanh)
                tc1 = ew.tile([128, 256], bf16, tag=f"tc1{c}", name="tc1")
                nc.scalar.activation(tc1[:], cb1[:], AF.Tanh)
                nc.vector.tensor_tensor(h_new[0][:], og[:, 0:256], tc0[:], OP.mult)
                nc.gpsimd.tensor_tensor(h_new[1][:], og[:, 256:512], tc1[:], OP.mult)

            for s in range(S):
                for c in range(2):
                    frot = fro_ps.tile([128, 1536], f32, tag="fro", name="fro")
                    ut = u_ps.tile([128, 512], f32, tag="ut", name="ut")

                    # one accumulation group per PSUM bank; per 4-gt gate group,
                    # k-outer over h chunks with the x chunk right after, so each
                    # gate's pre-activations finish as early as possible
                    def gate_group(gts):
                        for k in range(4):
                            for gt in gts:
                                nc.tensor.matmul(
                                    frot[:, gt * 128 : (gt + 1) * 128],
                                    lhsT=w_tile(k, gt),
                                    rhs=h_chunk(c, k),
                                    start=(k == 0 and gt % 4 == 0),
                                    stop=False,
                                )
                        for gt in gts:
                            nc.tensor.matmul(
                                frot[:, gt * 128 : (gt + 1) * 128],
                                lhsT=w_tile(4, gt),
                                rhs=x_chunk(c, s),
                                start=False,
                                stop=(gt % 4 == 3),
                            )

                    # ---- f then r gate matmuls ----
                    gate_group([0, 1, 2, 3])
                    gate_group([4, 5, 6, 7])

                    # ---- head activations + g-chain (can start mid-burst) ----
                    fg = ew.tile([128, 512], bf16, tag=f"fg{c}", name="fg")
                    nc.scalar.activation(fg[:], frot[:, 0:512], AF.Sigmoid)
                    rg = ew.tile([128, 512], bf16, tag=f"rg{c}", name="rg")
                    nc.scalar.activation(rg[:], frot[:, 512:1024], AF.Sigmoid)

                    p = ew.tile([128, 512], bf16, tag=f"p{c}", name="p")
                    m = ew.tile([128, 512], bf16, tag=f"m{c}", name="m")
                    rg2 = ew.tile([128, 512], bf16, tag=f"rg2{c}", name="rg2")
                    e2 = ew.tile([128, 512], bf16, tag=f"e2{c}", name="e2")
                    g2 = ew.tile([128, 512], bf16, tag=f"g{c}", name="g2")
                    nc.vector.tensor_tensor(p[:], fg[:], fg[:], OP.mult)
                    nc.vector.tensor_tensor(m[:], fg[:], p[:], OP.subtract)
                    nc.vector.tensor_scalar_mul(rg2[:], rg[:], 2.0)
                    nc.vector.tensor_tensor(e2[:], rg2[:], m[:], OP.mult)
                    nc.vector.tensor_tensor(g2[:], e2[:], p[:], OP.add)

                    # a = g*c can run before tanh(u) is even ready
                    gneg = ew.tile([128, 512], bf16, tag=f"gneg{c}", name="gneg")
                    nc.vector.tensor_scalar(gneg[:], g2[:], -1.0, 1.0, OP.mult, OP.add)
                    a = ew.tile([128, 512], f32, tag=f"a{c}", name="a")
                    nc.vector.tensor_tensor(a[:, 0:256], g2[:, 0:256], cbuf[c][0][:], OP.mult)
                    nc.gpsimd.tensor_tensor(a[:, 256:512], g2[:, 256:512], cbuf[c][1][:], OP.mult)

                    # ---- lagged y projection (lands in the round-old u bank;
                    # must be emitted before this step's u matmuls reuse it) ----
                    while len(pend_y) > 1:
                        emit_y_mms(*pend_y.pop(0))

                    # ---- u gate matmuls + tanh(u) ----
                    for gt in range(12, 16):
                        col = (gt - 12) * 128
                        for k in range(KCH):
                            nc.tensor.matmul(
                                ut[:, col : col + 128],
                                lhsT=w_tile(k, gt),
                                rhs=h_chunk(c, k) if k < 4 else x_chunk(c, s),
                                start=(gt == 12 and k == 0),
                                stop=(gt == 15 and k == KCH - 1),
                            )
                    tu = ew.tile([128, 512], bf16, tag=f"tu{c}", name="tu")
                    nc.scalar.activation(tu[:], ut[:], AF.Tanh)

                    # ---- cell update c' = a + (1-g)*tu: only b and the add
                    # remain serial after tanh(u); half 0 vector, half 1 gpsimd
                    b = ew.tile([128, 512], bf16, tag=f"b{c}", name="b")
                    nc.vector.tensor_tensor(b[:, 0:256], gneg[:, 0:256], tu[:, 0:256], OP.mult)
                    nc.vector.tensor_tensor(cbuf[c][0][:], a[:, 0:256], b[:, 0:256], OP.add)
                    nc.gpsimd.tensor_tensor(b[:, 256:512], gneg[:, 256:512], tu[:, 256:512], OP.mult)
                    nc.gpsimd.tensor_tensor(cbuf[c][1][:], a[:, 256:512], b[:, 256:512], OP.add)

                    # ---- deferred tail of the previous chain-step ----
                    emit_tail()

                    # ---- o gate matmuls + sigmoid(o) ----
                    gate_group([8, 9, 10, 11])
                    og = ew.tile([128, 512], bf16, tag=f"og{c}", name="og")
                    nc.scalar.activation(og[:], frot[:, 1024:1536], AF.Sigmoid)

                    h_new = [
                        hpool.tile([128, 256], bf16, tag=f"h{c}h{half}", name=f"hn{half}")
                        for half in range(2)
                    ]
                    pend_tail.append((c, cbuf[c][0], cbuf[c][1], og, h_new))
                    pend_y.append((h_new, x_chunk(c, s), 2 * s + c, ut))
                    h_prev[c] = h_new

            # flush deferred work
            while pend_tail:
                emit_tail()
            while pend_y:
                emit_y_mms(*pend_y.pop(0))

    nc.compile()
    return nc


# gate-block permutation: [f r o u] (orig order is [f r u o])
_PERM = np.concatenate(
    [np.arange(0, 1024), np.arange(1536, 2048), np.arange(1024, 1536)]
)


def _prep(inputs):
    x = np.asarray(inputs["x"], np.float32)
    W_ih = np.asarray(inputs["W_ih"], np.float32)
    W_hh = np.asarray(inputs["W_hh"], np.float32)
    b = np.asarray(inputs["b"], np.float32)
    fb = np.asarray(inputs["fb"], np.float32)
    W_out = np.asarray(inputs["W_out"], np.float32)
    b_out = np.asarray(inputs["b_out"], np.float32)
    bf = ml_dtypes.bfloat16

    bias_col = b.copy()
    bias_col[0:H] += fb
    bias_col[H : 2 * H] -= fb

    extra = np.zeros((128, G4), np.float32)
    extra[0:I] = W_ih.T
    extra[I] = bias_col
    Wfull = np.concatenate([W_hh.T, extra], axis=0)  # [640, 2048]
    Wfull = Wfull[:, _PERM]  # reorder gate blocks to [f r o u]
    w_host = (
        Wfull.reshape(KCH, 128, GT, 128).transpose(1, 0, 2, 3).reshape(128, -1)
    ).astype(bf)

    extra_y = np.zeros((128, 10), np.float32)
    extra_y[I] = b_out
    Wyfull = np.concatenate([W_out.T, extra_y], axis=0)  # [640, 10]
    wy_host = Wyfull.reshape(KCH, 128, 10).transpose(1, 0, 2).reshape(128, -1).astype(bf)

    xc = []
    for j in range(NCHUNK):
        start = j * C_OUT
        xs = x[:, start : start + S_STEPS, :]  # [128, S, 10]
        arr = np.zeros((128, S_STEPS * 128), np.float32)
        arr[0:I] = xs.transpose(2, 1, 0).reshape(I, -1)
        arr[I] = 1.0
        xc.append(arr.astype(bf))
    return w_host, wy_host, xc


def kernel(**inputs):
    from concourse.bass_utils import run_bass_kernel_spmd

    if "nc" not in _cache:
        _cache["nc"] = _build_nc(S_STEPS)
    nc = _cache["nc"]

    w_host, wy_host, xc = _prep(inputs)
    in_maps = []
    for core in range(NCORES):
        in_maps.append(
            {
                "w": w_host,
                "wy": wy_host,
                "x0": xc[2 * core],
                "x1": xc[2 * core + 1],
            }
        )
    res = run_bass_kernel_spmd(nc, in_maps, list(range(NCORES))).results

    y = np.zeros((B, T, 10), np.float32)
    for j in range(NCHUNK):
        core, chain = j // 2, j % 2
        ya = np.asarray(res[core]["y"], np.float32)  # [10, 2*S*128]
        yj = ya.reshape(10, S_STEPS, 2, 128)[:, :, chain, :]  # [10, S, 128]
        yj = yj.transpose(2, 1, 0)  # [B, S, 10]
        if j == 0:
            y[:, 0:S_STEPS, :] = yj
        else:
            start = j * C_OUT + W_WARM
            y[:, start : start + C_OUT, :] = yj[:, W_WARM:, :]
    return y
